# revision 20
# baseline (speedup 1.0000x reference)
"""MAAC critic kernel for Trainium2, data-parallel over the batch dim on 8 cores.

Self-contained: hardcodes all shapes. The harness calls kernel(**inputs) with the
full (unsharded) inputs and gets the full [A, B, 1] output back.

Per-core dataflow (B_local=2048, chunks of 256, b-tiles of 128):
  feature-major encoder (weights-stationary matmuls, fused bias+lrelu eviction),
  emitted as dense all-agent PE bursts so the tensor engine stays out of its
  low p-states
  -> b-major keys/sels (one fused two-run eviction) and vals evicted directly
     into a j-innermost transposed layout (vals bias via K=1 ones-row matmul)
  -> attention with wide 2x-mode DVE ops: one broadcast-AP product per engine
     slice, in-place halving trees over d, softmax with self-mask via a
     diagonal memset + exp on ScalarE, j-innermost weighted sum over agents
  -> critic MLP feature-major (attended values transposed back via PE
     transposes + paired ScalarE evictions) -> q-value PE transpose
  -> first-argmax one-hot gather of the taken action on GPSIMD (exact
     tie-breaking via a reversed-iota trick).
"""
import os
import numpy as np
import ml_dtypes

import concourse.bass as bass
import concourse.tile as tile
import concourse.mybir as mybir
from concourse import bacc
from concourse import bass_utils

F32 = mybir.dt.float32
BF16 = mybir.dt.bfloat16
bfloat16 = ml_dtypes.bfloat16

A = 8
B = 16384
S = 456
NACT = 16
IDIM = S + NACT          # 472
KPAD = 512               # padded encoder contraction dim
H = 256
E = 4
D = 64
ED = E * D               # 256
N_CORES = 8
B_LOCAL = B // N_CORES   # 2048
NB = 256                 # chunk size along b
ALPHA = 0.01             # leaky relu slope

# GPSIMD offload: phase-1 (prod + tree L1) for agents [GP_I0, A); phase-2
# weighted-sum product for agents in GP_P2. The argmax gather also runs on
# GPSIMD. Everything else runs on DVE.
GP_I0 = int(os.environ.get("MAAC_GP_I0", "7"))
GP_P2 = tuple(range(int(os.environ.get("MAAC_GP_P2", "7")), A))


def _ap(base, dims):
    """AP with base's partition dim + explicit free [step, count] dims (elements)."""
    return bass.AP(tensor=base.tensor, offset=base.offset, ap=[base.ap[0], *dims])


def build_bass(b_local=B_LOCAL):
    """Build and compile the single-core Bass module (same NEFF on all cores)."""
    nc = bacc.Bacc("TRN2", target_bir_lowering=False, debug=False)
    n_chunks = b_local // NB
    LRELU = (mybir.ActivationFunctionType.Relu
             if os.environ.get("MAAC_RELU") else mybir.ActivationFunctionType.Lrelu)
    EXP = mybir.ActivationFunctionType.Exp
    COPY = mybir.ActivationFunctionType.Copy
    MULT = mybir.AluOpType.mult
    ADD = mybir.AluOpType.add
    ISGE = mybir.AluOpType.is_ge
    ISEQ = mybir.AluOpType.is_equal
    MAX = mybir.AluOpType.max
    X = mybir.AxisListType.X
    NV = A - GP_I0           # agents on GPSIMD for phase 1
    ND = GP_I0               # agents on DVE for phase 1
    ISCALE = 1.0 / np.sqrt(np.float32(D))

    # ---- DRAM I/O ----
    saT_d = nc.dram_tensor("saT", [A, KPAD, b_local], BF16, kind="ExternalInput")
    acts_d = nc.dram_tensor("acts", [b_local, A, NACT], F32, kind="ExternalInput")
    w_enc_d = nc.dram_tensor("w_enc", [A, KPAD, 2 * H], BF16, kind="ExternalInput")
    b_enc_d = nc.dram_tensor("b_enc", [A, 4, 128, 1], F32, kind="ExternalInput")
    w_kvs_d = nc.dram_tensor("w_kvs", [H, 3 * H], BF16, kind="ExternalInput")
    kvbias_d = nc.dram_tensor("kvbias", [1, H], BF16, kind="ExternalInput")
    w_c1_d = nc.dram_tensor("w_c1", [A, 2 * H, H], BF16, kind="ExternalInput")
    b_c1_d = nc.dram_tensor("b_c1", [A, 2, 128, 1], F32, kind="ExternalInput")
    w_c2_d = nc.dram_tensor("w_c2", [A, H, NACT], BF16, kind="ExternalInput")
    b_c2_d = nc.dram_tensor("b_c2", [A, NACT, 1], F32, kind="ExternalInput")
    q_d = nc.dram_tensor("q", [A, b_local], F32, kind="ExternalOutput")

    with tile.TileContext(nc) as tc:
        import contextlib
        with contextlib.ExitStack() as ctx:
            wp = ctx.enter_context(tc.tile_pool(name="wp", bufs=1))
            xp = ctx.enter_context(tc.tile_pool(name="xp", bufs=3))
            ep = ctx.enter_context(tc.tile_pool(name="ep", bufs=2))
            sp = ctx.enter_context(tc.tile_pool(name="sp", bufs=3))
            bmp = ctx.enter_context(tc.tile_pool(name="bmp", bufs=2))
            scp = ctx.enter_context(tc.tile_pool(name="scp", bufs=1))
            scg = ctx.enter_context(tc.tile_pool(name="scg", bufs=1))
            ap_ = ctx.enter_context(tc.tile_pool(name="ap", bufs=2))
            op_ = ctx.enter_context(tc.tile_pool(name="op", bufs=2))
            cp = ctx.enter_context(tc.tile_pool(name="cp", bufs=3))
            pbig = ctx.enter_context(tc.tile_pool(name="pbig", bufs=2, space="PSUM"))
            pmed = ctx.enter_context(tc.tile_pool(name="pmed", bufs=2, space="PSUM"))
            ptr = ctx.enter_context(tc.tile_pool(name="ptr", bufs=2, space="PSUM"))

            # ---- resident weights ----
            w_enc_sb, b_enc_sb = [], []
            w_c1_sb, b_c1_sb = [], []
            w_c2_sb, b_c2_sb = [], []
            for a in range(A):
                w = wp.tile([128, 4, 2 * H], BF16, name=f"w_enc{a}")
                nc.scalar.dma_start(out=w, in_=w_enc_d[a].rearrange("(kt p) m -> p kt m", p=128))
                w_enc_sb.append(w)
                bt_ = wp.tile([128, 4, 1], F32, name=f"b_enc{a}")
                nc.scalar.dma_start(out=bt_, in_=b_enc_d[a].rearrange("kt p one -> p kt one"))
                b_enc_sb.append(bt_)
                w1 = wp.tile([128, 4, H], BF16, name=f"w_c1{a}")
                nc.scalar.dma_start(out=w1, in_=w_c1_d[a].rearrange("(kt p) m -> p kt m", p=128))
                w_c1_sb.append(w1)
                b1 = wp.tile([128, 2, 1], F32, name=f"b_c1{a}")
                nc.scalar.dma_start(out=b1, in_=b_c1_d[a].rearrange("kt p one -> p kt one"))
                b_c1_sb.append(b1)
                w2 = wp.tile([128, 2, NACT], BF16, name=f"w_c2{a}")
                nc.scalar.dma_start(out=w2, in_=w_c2_d[a].rearrange("(kt p) m -> p kt m", p=128))
                w_c2_sb.append(w2)
                b2 = wp.tile([NACT, 1], F32, name=f"b_c2{a}")
                nc.scalar.dma_start(out=b2, in_=b_c2_d[a])
                b_c2_sb.append(b2)
            w_kvs_sb = wp.tile([128, 2, 3 * H], BF16, name="w_kvs")
            nc.scalar.dma_start(out=w_kvs_sb, in_=w_kvs_d.rearrange("(kt p) m -> p kt m", p=128))
            kvbias_sb = wp.tile([1, H], BF16, name="kvbias")
            nc.scalar.dma_start(out=kvbias_sb, in_=kvbias_d[:, :])
            ones_sb = wp.tile([1, 128], BF16, name="ones")
            nc.vector.memset(ones_sb, 1.0)
            from concourse.masks import make_identity
            ident16 = wp.tile([16, 16], BF16, name="ident16")
            make_identity(nc, ident16)
            ident128 = wp.tile([128, 128], BF16, name="ident128")
            make_identity(nc, ident128)
            # reversed iota (16 - o) per action slot, for first-argmax ties
            iota_rev = wp.tile([128, A, NACT], F32, name="iota_rev")
            nc.gpsimd.iota(iota_rev, pattern=[[0, A], [-1, NACT]], base=NACT,
                           channel_multiplier=0,
                           allow_small_or_imprecise_dtypes=True)

            nbt = NB // 128  # b-tiles per chunk (2)
            ct = {}          # per-chunk live tiles, keyed by chunk index

            # ---------- phase: feature-major encoder + b-major kvs ----------
            def emit_encoder(c):
                c0 = c * NB
                s_enc_all = sp.tile([128, A, 2, NB], BF16, tag="s_enc")
                # ks_bm plane 0 = keys, plane 1 = sels (shared so one ACT evicts both)
                ks_bm = [bmp.tile([128, 2, A, ED], BF16, tag=f"ks{bt}", name=f"ks{bt}")
                         for bt in range(nbt)]
                # vals transposed: [ (e), (d), j ] with the agent axis innermost
                valsT = [bmp.tile([128, E, D, A], BF16, tag=f"vt{bt}", name=f"vt{bt}")
                         for bt in range(nbt)]
                ct[c] = dict(s_enc_all=s_enc_all, ks_bm=ks_bm, valsT=valsT)
                # per-agent interleave: enc chains then that agent's kvs, so
                # the keys/sels evictions land incrementally through the phase
                # (the j-half-split ph1 product can then start mid-phase)
                for a in range(A):
                    x = xp.tile([128, 4, NB], BF16, tag="x")
                    nc.sync.dma_start(
                        out=x,
                        in_=saT_d[a].rearrange("(kt p) b -> p kt b", p=128)[:, :, c0:c0 + NB])
                    sa_enc = ep.tile([128, 2, NB], BF16, tag="sa_enc")
                    for mt in range(4):
                        ps = pmed.tile([128, NB], F32, tag="mm")
                        for kt in range(4):
                            nc.tensor.matmul(out=ps,
                                             lhsT=w_enc_sb[a][:, kt, mt * 128:(mt + 1) * 128],
                                             rhs=x[:, kt, :],
                                             start=(kt == 0), stop=(kt == 3))
                        dst = sa_enc[:, mt % 2, :] if mt < 2 else s_enc_all[:, a, mt - 2, :]
                        nc.scalar.activation(out=dst, in_=ps, func=LRELU,
                                             bias=b_enc_sb[a][:, mt, :],
                                             scale=1.0, alpha=ALPHA)

                    # b-major keys/vals/sels for this agent
                    # psum cols 0:256 = keys, 256:512 = vals (+bias), 512:768 = sels
                    for bt in range(nbt):
                        ps2 = pbig.tile([128, 1024], F32, tag="kv")
                        for kt in range(2):
                            nc.tensor.matmul(out=ps2[:, 0:H],
                                             lhsT=sa_enc[:, kt, bt * 128:(bt + 1) * 128],
                                             rhs=w_kvs_sb[:, kt, 0:H],
                                             start=(kt == 0), stop=(kt == 1))
                        for kt in range(2):
                            nc.tensor.matmul(out=ps2[:, H:2 * H],
                                             lhsT=sa_enc[:, kt, bt * 128:(bt + 1) * 128],
                                             rhs=w_kvs_sb[:, kt, H:2 * H],
                                             start=(kt == 0), stop=False)
                        nc.tensor.matmul(out=ps2[:, H:2 * H], lhsT=ones_sb, rhs=kvbias_sb,
                                         start=False, stop=True)
                        for kt in range(2):
                            nc.tensor.matmul(out=ps2[:, 2 * H:3 * H],
                                             lhsT=s_enc_all[:, a, kt, bt * 128:(bt + 1) * 128],
                                             rhs=w_kvs_sb[:, kt, 2 * H:3 * H],
                                             start=(kt == 0), stop=(kt == 1))
                        # contiguous single-run evictions (a two-run fused ACT
                        # measures ~3x slower than two one-run ACTs)
                        nc.scalar.activation(
                            out=ks_bm[bt][:, 0, a, :], in_=ps2[:, 0:ED],
                            func=COPY, bias=0.0, scale=1.0)
                        nc.scalar.activation(
                            out=ks_bm[bt][:, 1, a, :], in_=ps2[:, 2 * H:2 * H + ED],
                            func=COPY, bias=0.0, scale=1.0)
                        # vals with lrelu, straight into the j-innermost layout
                        nc.scalar.activation(
                            out=_ap(valsT[bt][:, 0, 0, a:a + 1], [[D * A, E], [A, D]]),
                            in_=_ap(ps2[:, ED:ED + 1], [[1, ED]]),
                            func=LRELU, bias=0.0, scale=1.0, alpha=ALPHA)

            # ---------- attention phase 1: logits + softmax probabilities ----------
            def emit_attention_ph1(c):
                ks_bm, valsT = ct[c]["ks_bm"], ct[c]["valsT"]
                ct[c]["p2"] = []
                for bt in range(nbt):
                    keys = ks_bm[bt][:, 0]
                    sels = ks_bm[bt][:, 1]
                    scr_v = scp.tile([128, ND, A, ED], BF16, tag="scr_v", name="scr_v")
                    scr_g = scg.tile([128, NV, A, ED], BF16, tag=f"scr_g{bt}", name=f"scr_g{bt}")

                    # phase-1 products: prod[i, j, (e d)] = sels[i] * keys[j],
                    # split by j-half so the first half can start as soon as
                    # agents 0..3's keys/sels are evicted (mid encoder phase)
                    if NV:
                        nc.gpsimd.tensor_tensor(
                            out=_ap(scr_g[:, 0, 0, 0:1], [[A * ED, NV], [ED, A], [1, ED]]),
                            in0=_ap(sels[:, GP_I0, 0:1], [[ED, NV], [0, A], [1, ED]]),
                            in1=_ap(keys[:, 0, 0:1], [[0, NV], [ED, A], [1, ED]]),
                            op=MULT)
                    half = A // 2
                    for j0 in (0, half):
                        nc.vector.tensor_tensor(
                            out=_ap(scr_v[:, 0, j0, 0:1], [[A * ED, ND], [ED, half], [1, ED]]),
                            in0=_ap(sels[:, 0, 0:1], [[ED, ND], [0, half], [1, ED]]),
                            in1=_ap(keys[:, j0, 0:1], [[0, ND], [ED, half], [1, ED]]),
                            op=MULT)

                    # d-tree, in place: level h sums [0:h] += [h:2h] within each (i,j,e)
                    def tree_level(eng, scr, nij, h):
                        eng.tensor_tensor(
                            out=_ap(scr[:, 0, 0, 0:1], [[ED, nij], [D, E], [1, h]]),
                            in0=_ap(scr[:, 0, 0, 0:1], [[ED, nij], [D, E], [1, h]]),
                            in1=_ap(scr[:, 0, 0, h:h + 1], [[ED, nij], [D, E], [1, h]]),
                            op=ADD)
                    if NV:
                        tree_level(nc.gpsimd, scr_g, NV * A, 32)
                    # all DVE-side levels first, then the GPSIMD-side ones, so the
                    # DVE never head-of-line blocks on the slower GPSIMD L1
                    for h in (32, 16, 8, 4, 2):
                        tree_level(nc.vector, scr_v, ND * A, h)
                    for h in (16, 8, 4, 2):
                        if NV:
                            tree_level(nc.vector, scr_g, NV * A, h)

                    # final level -> l_t[i, e, j] fp32
                    l_t = ap_.tile([128, A, E, A], F32, tag="l")
                    for scr, i0, ni in ((scr_v, 0, ND), (scr_g, GP_I0, NV)):
                        if not ni:
                            continue
                        nc.vector.tensor_tensor(
                            out=_ap(l_t[:, i0, 0, 0:1], [[E * A, ni], [1, A], [A, E]]),
                            in0=_ap(scr[:, 0, 0, 0:1], [[A * ED, ni], [ED, A], [D, E]]),
                            in1=_ap(scr[:, 0, 0, 1:2], [[A * ED, ni], [ED, A], [D, E]]),
                            op=ADD)
                    # self-mask: l[i, :, j=i] = -1e9 (exp underflows to 0)
                    nc.vector.memset(
                        _ap(l_t[:, 0, 0, 0:1], [[E * A + 1, A], [A, E]]), -1e9)

                    # softmax over j; 1/sqrt(D) folded into exp scale
                    wexp = ap_.tile([128, A, E, A], BF16, tag="wexp")
                    nc.scalar.activation(out=wexp.rearrange("p i e j -> p (i e j)"),
                                         in_=l_t.rearrange("p i e j -> p (i e j)"),
                                         func=EXP, scale=ISCALE)
                    ssum = ap_.tile([128, A * E], F32, tag="ssum")
                    nc.vector.tensor_reduce(out=ssum,
                                            in_=wexp.rearrange("p i e j -> p (i e) j"),
                                            axis=X, op=ADD)
                    rs = ap_.tile([128, A * E], F32, tag="rs")
                    nc.vector.reciprocal(out=rs, in_=ssum)
                    # p2[i, e, j] = wexp[i, e, j] * rs[i, e]
                    p2 = ap_.tile([128, A, E, A], BF16, tag="p2")
                    nc.vector.tensor_tensor(
                        out=_ap(p2[:, 0, 0, 0:1], [[E * A, A], [A, E], [1, A]]),
                        in0=_ap(wexp[:, 0, 0, 0:1], [[E * A, A], [A, E], [1, A]]),
                        in1=_ap(rs[:, 0:1], [[E, A], [1, E], [0, A]]),
                        op=MULT)
                    ct[c]["p2"].append(p2)

            # ---------- attention phase 2: probability-weighted value sum ----------
            def emit_attention_ph2(c):
                valsT = ct[c]["valsT"]
                other_bm = [op_.tile([128, A, ED], BF16, tag=f"other{bt}", name=f"other{bt}")
                            for bt in range(nbt)]
                ct[c]["other_bm"] = other_bm
                for bt in range(nbt):
                    p2 = ct[c]["p2"][bt]
                    scr_v = scp.tile([128, ND, A, ED], BF16, tag="scr_v", name="scr_v")
                    scr_g = scg.tile([128, NV, A, ED], BF16, tag=f"scr_g{bt}", name=f"scr_g{bt}")

                    # products: scr[i] <- [e, d, j] = p2[i, e, j] * valsT[e, d, j]
                    for i in range(A):
                        scr, io = (scr_v, i) if i < GP_I0 else (scr_g, i - GP_I0)
                        eng = nc.gpsimd if i in GP_P2 else nc.vector
                        eng.tensor_tensor(
                            out=_ap(scr[:, io, 0, 0:1], [[D * A, E], [A, D], [1, A]]),
                            in0=_ap(p2[:, i, 0, 0:1], [[A, E], [0, D], [1, A]]),
                            in1=_ap(valsT[bt][:, 0, 0, 0:1], [[D * A, E], [A, D], [1, A]]),
                            op=MULT)

                    # j-tree (in place over the innermost agent axis) then final
                    for scr, i0, ni in ((scr_v, 0, ND), (scr_g, GP_I0, NV)):
                        if not ni:
                            continue
                        for h in (4, 2):
                            nc.vector.tensor_tensor(
                                out=_ap(scr[:, 0, 0, 0:1], [[2048, ni], [A, ED], [1, h]]),
                                in0=_ap(scr[:, 0, 0, 0:1], [[2048, ni], [A, ED], [1, h]]),
                                in1=_ap(scr[:, 0, 0, h:h + 1], [[2048, ni], [A, ED], [1, h]]),
                                op=ADD)
                        nc.vector.tensor_tensor(
                            out=_ap(other_bm[bt][:, i0, 0:1], [[ED, ni], [1, ED]]),
                            in0=_ap(scr[:, 0, 0, 0:1], [[2048, ni], [A, ED]]),
                            in1=_ap(scr[:, 0, 0, 1:2], [[2048, ni], [A, ED]]),
                            op=ADD)

            # ---------- phase: critic (per agent) + argmax gather ----------
            def emit_critic_gather(c):
                c0 = c * NB
                s_enc_all = ct[c]["s_enc_all"]
                other_bm = ct[c]["other_bm"]
                # allq in b-major: [128, bt, a, NACT] fp32, one tile per chunk
                allq_bm = op_.tile([128, nbt, A, NACT], F32, tag="allq", name="allq")
                for a in range(A):
                    # transposes of the attended values, paired per kt into one
                    # psum bank so each eviction moves 256 elems
                    otherT = cp.tile([128, 2, NB], BF16, tag="otherT")
                    for kt in range(2):
                        pt = ptr.tile([128, 2, 128], BF16, tag="tr")
                        for bt in range(nbt):
                            nc.tensor.transpose(
                                out=pt[:, bt, :],
                                in_=other_bm[bt][:, a, kt * 128:(kt + 1) * 128],
                                identity=ident128)
                        nc.scalar.activation(
                            out=otherT[:, kt, :],
                            in_=pt.rearrange("p b x -> p (b x)"),
                            func=COPY, bias=0.0, scale=1.0)
                    h_t = cp.tile([128, 2, NB], BF16, tag="h")
                    for mt in range(2):
                        ps = pmed.tile([128, NB], F32, tag="mm")
                        for kt in range(4):
                            rhs = s_enc_all[:, a, kt, :] if kt < 2 else otherT[:, kt - 2, :]
                            nc.tensor.matmul(out=ps,
                                             lhsT=w_c1_sb[a][:, kt, mt * 128:(mt + 1) * 128],
                                             rhs=rhs,
                                             start=(kt == 0), stop=(kt == 3))
                        nc.scalar.activation(out=h_t[:, mt, :], in_=ps,
                                             func=LRELU, bias=b_c1_sb[a][:, mt, :],
                                             scale=1.0, alpha=ALPHA)
                    psq = pmed.tile([128, NB], F32, tag="mm")
                    for kt in range(2):
                        nc.tensor.matmul(out=psq[0:NACT, :], lhsT=w_c2_sb[a][:, kt, :],
                                         rhs=h_t[:, kt, :],
                                         start=(kt == 0), stop=(kt == 1))
                    allq = cp.tile([NACT, NB], BF16, tag="allq")
                    nc.scalar.activation(out=allq, in_=psq[0:NACT, :],
                                         func=mybir.ActivationFunctionType.Identity,
                                         bias=b_c2_sb[a], scale=1.0)
                    ptq = ptr.tile([128, 2, NACT], BF16, tag="tr")
                    for bt in range(nbt):
                        nc.tensor.transpose(out=ptq[:, bt, :],
                                            in_=allq[:, bt * 128:(bt + 1) * 128],
                                            identity=ident16)
                    nc.scalar.activation(
                        out=_ap(allq_bm[:, 0, a, 0:1], [[A * NACT, 2], [1, NACT]]),
                        in_=ptq.rearrange("p b x -> p (b x)"),
                        func=COPY, bias=0.0, scale=1.0)

                # ---------- first-argmax gather on GPSIMD (per b-tile) ----------
                for bt in range(nbt):
                    acts_t = op_.tile([128, A, NACT], F32, tag="acts")
                    nc.sync.dma_start(out=acts_t, in_=acts_d[c0 + bt * 128: c0 + (bt + 1) * 128])
                    amax = op_.tile([128, A], F32, tag="amax")
                    nc.vector.tensor_reduce(out=amax, in_=acts_t, axis=X, op=MAX)
                    # mi[o] = (acts[o] >= amax) * (16 - o); max(mi) hits the
                    # FIRST tied argmax (largest reversed index)
                    mi = op_.tile([128, A, NACT], F32, tag="mi")
                    nc.vector.tensor_tensor(
                        out=mi.rearrange("p a o -> p (a o)"),
                        in0=acts_t.rearrange("p a o -> p (a o)"),
                        in1=_ap(amax[:, 0:1], [[1, A], [0, NACT]]),
                        op=ISGE)
                    nc.gpsimd.tensor_tensor(out=mi.rearrange("p a o -> p (a o)"),
                                            in0=mi.rearrange("p a o -> p (a o)"),
                                            in1=iota_rev.rearrange("p a o -> p (a o)"),
                                            op=MULT)
                    rmax = op_.tile([128, A], F32, tag="rmax")
                    nc.vector.tensor_reduce(out=rmax, in_=mi, axis=X, op=MAX)
                    onehot = op_.tile([128, A, NACT], F32, tag="onehot")
                    nc.vector.tensor_tensor(
                        out=onehot.rearrange("p a o -> p (a o)"),
                        in0=mi.rearrange("p a o -> p (a o)"),
                        in1=_ap(rmax[:, 0:1], [[1, A], [0, NACT]]),
                        op=ISEQ)
                    nc.gpsimd.tensor_tensor(
                        out=onehot.rearrange("p a o -> p (a o)"),
                        in0=onehot.rearrange("p a o -> p (a o)"),
                        in1=allq_bm[:, bt].rearrange("p a o -> p (a o)"),
                        op=MULT)
                    q_sb = op_.tile([128, A], F32, tag="qsb")
                    nc.vector.tensor_reduce(out=q_sb, in_=onehot, axis=X, op=ADD)
                    nc.sync.dma_start(
                        out=bass.AP(tensor=q_d, offset=c0 + bt * 128,
                                    ap=[[1, 128], [b_local, A]]),
                        in_=q_sb)
                del ct[c]

            # ---------- software pipeline ----------
            # Per step: softmax of chunk c-1 first (so its exp isn't queued
            # behind the next encoder's evictions on ScalarE), then the next
            # encoder (keeps PE busy during attention), then the value sum and
            # critic of chunk c-1.
            emit_encoder(0)
            for c in range(1, n_chunks):
                emit_attention_ph1(c - 1)
                emit_encoder(c)
                emit_attention_ph2(c - 1)
                emit_critic_gather(c - 1)
            emit_attention_ph1(n_chunks - 1)
            emit_attention_ph2(n_chunks - 1)
            emit_critic_gather(n_chunks - 1)

    nc.compile()
    return nc


def _prep_inputs(states, actions, enc_W, enc_b, s_W, s_b, key_W, sel_W,
                 val_W, val_b, c_W1, c_b1, c_W2, c_b2,
                 b_local=B_LOCAL, n_cores=N_CORES):
    """Host-side: build per-core input dicts (shard over B, bf16 layouts)."""
    f32 = np.float32
    Bv = b_local * n_cores
    states = states[:, :Bv]
    actions = actions[:, :Bv]
    sa = np.concatenate([states, actions], axis=-1).astype(f32)      # [A, Bv, 472]
    saT = np.zeros((A, KPAD, Bv), dtype=bfloat16)
    saT[:, :IDIM, :] = sa.transpose(0, 2, 1).astype(bfloat16)
    w_enc = np.zeros((A, KPAD, 2 * H), dtype=bfloat16)
    w_enc[:, :IDIM, :H] = enc_W.astype(bfloat16)
    w_enc[:, :S, H:] = s_W.astype(bfloat16)
    b_enc = np.concatenate([enc_b, s_b], axis=-1).astype(f32).reshape(A, 4, 128, 1)
    w_kvs = np.zeros((H, 3 * H), dtype=bfloat16)
    w_kvs[:, 0:H] = key_W.transpose(1, 0, 2).reshape(H, H).astype(bfloat16)
    w_kvs[:, H:2 * H] = val_W.transpose(1, 0, 2).reshape(H, H).astype(bfloat16)
    w_kvs[:, 2 * H:] = sel_W.transpose(1, 0, 2).reshape(H, H).astype(bfloat16)
    kvbias = val_b.reshape(1, -1).astype(bfloat16)
    w_c1 = c_W1.astype(bfloat16)
    b_c1 = c_b1.astype(f32).reshape(A, 2, 128, 1)
    w_c2 = c_W2.astype(bfloat16)
    b_c2 = c_b2.astype(f32).reshape(A, NACT, 1)
    acts_bm = actions.transpose(1, 0, 2).astype(f32)                # [Bv, A, 16]

    shared = dict(w_enc=w_enc, b_enc=b_enc, w_kvs=w_kvs, kvbias=kvbias,
                  w_c1=w_c1, b_c1=b_c1, w_c2=w_c2, b_c2=b_c2)
    in_maps = []
    for cid in range(n_cores):
        sl = slice(cid * b_local, (cid + 1) * b_local)
        m_ = dict(shared)
        m_["saT"] = np.ascontiguousarray(saT[:, :, sl])
        m_["acts"] = np.ascontiguousarray(acts_bm[sl])
        in_maps.append(m_)
    return in_maps


_NC_CACHE = {}


def _get_nc(b_local=B_LOCAL):
    if b_local not in _NC_CACHE:
        _NC_CACHE[b_local] = build_bass(b_local)
    return _NC_CACHE[b_local]


def kernel(**inputs):
    inputs = {k: np.asarray(v) for k, v in inputs.items()}
    in_maps = _prep_inputs(**inputs)
    nc = _get_nc()
    res = bass_utils.run_bass_kernel_spmd(
        nc, in_maps, core_ids=list(range(N_CORES)),
        trace=bool(int(os.environ.get("MAAC_TRACE", "0"))))
    q = np.concatenate([r["q"] for r in res.results], axis=1)  # [A, B]
    if res.exec_time_ns is not None:
        print(f"HW exec time: {res.exec_time_ns} ns")
    return q[:, :, None].astype(np.float32)
